# revision 111
# baseline (speedup 1.0000x reference)
"""Single-head attention kernel for TRN2, 8 NeuronCores.

Problem: hidden [4,4096,1024] fp32; Wq/Wk/Wv [1024,64]; out [4,4096,64]
  q,k,v = hidden @ W + b ; out = softmax(q k^T / 8) @ v

Sharding: 2 cores per batch; each core handles 2048 query rows but computes
K/V for the full 4096-row sequence of its batch (sequence parallelism over
the Q rows, K/V recomputed per core — no collectives). Host-side prep per
core: hidden[b] is rotated so this core's query rows are rows 0:2048, cast
to bf16 and laid out [128, S, NT] so partition p holds hid[s, 128t+p].
Softmax over keys is permutation-invariant, so the rotation is sound.

Design (cost model: matmul = moving-cols x 1cyc/row for bf16 any width and
f32r >=256 wide; PE is HW-decoded so many small matmuls are cheap):
  proj:   psum[KV|Q, 512 seq] = sum_t W_tile[:,t,:]^T @ hid[:, s0:s0+512, t]
          (hid/weights bf16; the [p][s][t] host layout gives 8 contiguous
          128-descriptor DMAs instead of 8192 descriptors)
  kT/qT:  [64, seq] f32r via DVE bias-add (f32r keeps score precision; the
          512-wide moving side still runs 1 cyc/row)
  vones:  [128, NK, 65] bf16; cols 0:64 = v (via PE transpose), col 64 = 1
  scores: sc psum [128 keys, 1024 q] per (tile, pair): 2 matmuls ap=512
  exp:    one ACT instr per sc tile -> wt bf16 [128, 1024], scale=1/8
          (ACT is the bottleneck engine: 64 x 1024 cols at 1.2 GHz)
  AV:     q-partition orientation: acc[128 q, 65] += wt[:,128j:]^T @ vones
          (ap=65 bf16 - half the PE cycles of the [65, q] orientation;
          col 64 accumulates softmax denominators; acc banks are pre-zeroed
          on DVE and accumulated with start=False because a start=True in a
          bank wipes other open accumulations in that bank)
  out:    acc * reciprocal(denom): one stride-0-broadcast
          scalar_tensor_tensor per acc half (4 recips + 1 fused mul + 1
          DMA), A half first - its avs only need wt[:,0:512] of the split
          last exp, so its normalize+DMA overlap the B half's exp/flush.
  startup: DMA order = hid slivers (128/256 rows) first, then merged
          weight/bias tensors, then growing hid chunks; q/kv chunk-0
          projections run in 128-row quarters gated per-sliver; pass-0
          tiles 0-3 run A-QUARTER exps (256 wide) from ~6.9us, tiles 0-5
          split A/B at 512; junk warmup matmuls ramp the PE p-state.
          ACT is STRICT FIFO (no OOO window, unlike PE's 32) - head exps
          are emitted in gate-readiness order (t0/t1 A1+A2 before t2/t3,
          whose kT[256:512] chain lands ~1us later), else ready exps
          queue behind stalled ones.
  pieces: kv chunks 2-7 + qg2 + qg3 + c7's v pieces pop 2/tile (pre-sc +
          post-exp) from tile 6, 3 pops in the head's B window, a 3rd
          in-loop pop from t>=28; chunk c's k_add lands at tile 4c's
          pre-pop, just before its first reader.  The emission ORDER is
          the dependency order - a reader emitted before its writer gets
          no semaphore and races on HW; CoreSim catches that; engines
          dispatch out-of-order within their exec windows (PE 32), so
          among READY instructions order matters little.
  defer:  ALL pass-0 avs (plus its out_blocks) run inside pass 1's PE
          slack (pass 0's piece window is PE-oversubscribed); because of
          that, the accp banks are UNTOUCHED during pass 0 and serve as a
          third score buffer there (sc_alloc: every 3rd pass-0 sc tile,
          capped at alloc 36 so the [128,8,128] accAB accumulator tile
          can materialize at the pass boundary before the first deferred
          av).  PE running 2 sc tiles ahead of ACT absorbed all the
          chunk-boundary k_add-latency gaps (-2.2us).  pass-1's
          accumulators live in the pjp banks (idle by then); wtp bufs=36
          holds the unread wt tiles across the pass boundary.  All pieces
          MUST drain inside pass 0: anything later deadlocks the pjp
          rotation against acc1A/B's memsets.
  LAG=6 / DEFER=26 / 3rd-pop>=28 / cap-36 are TimelineSim-tuned; exp
  instr count (64x ~1024 wide) is the ACT floor at (N+222)/1.2 ns per
  instr -- 2048-wide exps need 8 PSUM banks of double-buffered scores
  and were not reachable even with the accp trick (pass 1 needs 4 acc
  banks live).
  DEAD ENDS (tried, failed): fp8e4m3 x16-scaled hid/weights for the
  projections -> rel err 3.2e-2 on CoreSim (gate 2e-2; exp amplifies the
  ~4% score error); nc.sync.dma_start_transpose for the v path -> CoreSim
  passes (1.9e-3) but real HW returns NaN, so the interp's model of
  InstDmaTransposeAnt diverges from silicon - do not trust it without a
  HW run.
"""

import numpy as np

E, S, H = 1024, 4096, 64
NT = E // 128  # 8 e-tiles
SQ = S // 2  # 2048 query rows per core
NK = S // 128  # 32 s_k tiles
N_CORES = 8
HEAD_T = 6  # startup tiles with split A/B exps
LAG = 6  # av trails exp by this many tiles
DEFER = 26  # pass-0 av groups before this tile run inside pass 1's slack

_NC = None
LAST_RESULT = None  # BassKernelResults of the most recent run (for test.py)


def _build(dbg=False):
    from contextlib import ExitStack
    import concourse.tile as tile
    from concourse import bacc, mybir
    from concourse.masks import make_identity

    F32 = mybir.dt.float32
    F32R = mybir.dt.float32r
    BF16 = mybir.dt.bfloat16
    Exp = mybir.ActivationFunctionType.Exp
    Copy = mybir.ActivationFunctionType.Copy

    nc = bacc.Bacc("TRN2", target_bir_lowering=False, debug=False)
    if dbg:
        QTD = nc.dram_tensor("qtd", [64, SQ], F32, kind="ExternalOutput")
        KTD = nc.dram_tensor("ktd", [64, S], F32, kind="ExternalOutput")
        VOD = nc.dram_tensor("vod", [128, NK * (H + 1)], F32, kind="ExternalOutput")
        ACCD = nc.dram_tensor("accd", [128, 4 * 128], F32, kind="ExternalOutput")
    HIDT = nc.dram_tensor("hidt", [128, S, NT], BF16, kind="ExternalInput")
    # all weights in one tensor: [:, :, 0:H]=Wq, [:, :, H:3H]=Wk|Wv
    WALL = nc.dram_tensor("wall", [128, NT, 3 * H], BF16, kind="ExternalInput")
    # biases in one tensor: col 0 = [bk;bv], col 1 = [bq; 0]
    BALL = nc.dram_tensor("ball", [128, 2], F32, kind="ExternalInput")
    OUT = nc.dram_tensor("out", [SQ, H], F32, kind="ExternalOutput")

    with tile.TileContext(nc) as tc, ExitStack() as ctx:
        consts = ctx.enter_context(tc.tile_pool(name="consts", bufs=1))
        hidp = ctx.enter_context(tc.tile_pool(name="hid", bufs=1))
        stage = ctx.enter_context(tc.tile_pool(name="stage", bufs=3))
        wtp = ctx.enter_context(tc.tile_pool(name="wt", bufs=36))
        dbgp = ctx.enter_context(tc.tile_pool(name="dbg", bufs=1)) if dbg else None
        scp = ctx.enter_context(tc.tile_pool(name="scp", bufs=2, space="PSUM"))
        pjp = ctx.enter_context(tc.tile_pool(name="pjp", bufs=2, space="PSUM"))
        accp = ctx.enter_context(tc.tile_pool(name="accp", bufs=1, space="PSUM"))

        # ---- constants / SBUF layout ----
        # wall: [:, t, 0:H]=Wq[t], [:, t, H:3H]=Wk|Wv[t]
        # ball: col 0 = [bk;bv], col 1 = [bq; 0]
        wall_sb = consts.tile([128, NT, 3 * H], BF16)
        ball_sb = consts.tile([128, 2], F32)
        identf = consts.tile([128, 128], F32)
        make_identity(nc, identf[:])
        identr = consts.tile([128, 128], F32R)
        nc.vector.tensor_copy(identr[:], identf[:])
        vones = consts.tile([128, NK, H + 1], BF16)
        ones32 = consts.tile([128, NK, 1], F32)
        nc.vector.memset(ones32[:], 1.0)
        nc.vector.tensor_copy(vones[:, :, 64:65], ones32[:])
        kT = consts.tile([64, S], F32R)
        qT = consts.tile([64, SQ], F32R)
        hidT_sb = hidp.tile([128, S, NT], BF16)

        # warm the Exp table early so the first real exp doesn't pay ~2.7us
        warm = consts.tile([1, 1], F32)
        nc.vector.memset(warm[:], 0.0)
        nc.scalar.activation(warm[:], warm[:], Exp)

        # warmup source for p-state ramp matmuls (zeros; results are junk)
        wsrc = consts.tile([128, 512], BF16)
        nc.vector.memset(wsrc[:], 0.0)

        # ---- DMAs on the sync/HWDGE queue (DMA engines are serial: order =
        # the startup chain).  Weights+biases are single merged DMAs; hid
        # chunk 0 split in half so projections start sooner.
        def dma_hid(c0, cols):
            nc.sync.dma_start(
                hidT_sb[:, c0 : c0 + cols, :], HIDT[:, c0 : c0 + cols, :]
            )

        dma_hid(0, 128)
        nc.sync.dma_start(wall_sb[:], WALL[:])
        dma_hid(128, 128)
        nc.sync.dma_start(ball_sb[:], BALL[:])
        dma_hid(256, 128)
        dma_hid(384, 128)
        dma_hid(512, 256)
        dma_hid(768, 256)
        for c in range(2, 8):
            dma_hid(512 * c, 512)

        # ---- PE warmup: ramp the p-state while DMA c0 lands ----
        for _ in range(7):
            junk = pjp.tile([128, 512], F32, tag="pj", name="warm")
            nc.tensor.matmul(junk[:], wsrc[:, 0:128], wsrc[:], start=True, stop=True)

        # ---- projection helpers ----
        def q_mm(box, g, t0, t1, h0=0, h1=512):
            # zeroed boxes (head/startup) use start=False so column-range
            # quarters run independently: start=True's bank-clear would
            # serialize quarter N+1 behind quarter N's reader
            zeroed = box.get("z", False)
            if "pq" not in box:
                box["pq"] = pjp.tile([64, 512], F32, tag="pj", name="pq")
                if zeroed:
                    nc.vector.memset(box["pq"][:], 0.0)
            for t in range(t0, t1):
                nc.tensor.matmul(
                    box["pq"][:, h0:h1],
                    wall_sb[:, t, 0:H],
                    hidT_sb[:, 512 * g + h0 : 512 * g + h1, t],
                    start=(t == 0 and not zeroed),
                    stop=(t == NT - 1),
                    skip_group_check=True,
                )

        def q_add(box, g, h0=0, h1=512):
            nc.vector.tensor_scalar_add(
                qT[:, 512 * g + h0 : 512 * g + h1],
                box["pq"][:, h0:h1],
                ball_sb[0:64, 1:2],
            )

        def q_group_pieces(g):
            box = {}
            return [
                lambda: q_mm(box, g, 0, 2),
                lambda: q_mm(box, g, 2, 4),
                lambda: q_mm(box, g, 4, 6),
                lambda: q_mm(box, g, 6, 8),
                lambda: q_add(box, g),
            ]

        def kv_mm(box, c, t0, t1, h0=0, h1=512):
            zeroed = box.get("z", False)
            if "pkv" not in box:
                box["pkv"] = pjp.tile([128, 512], F32, tag="pj", name="pkv")
                if zeroed:
                    nc.vector.memset(box["pkv"][:], 0.0)
            for t in range(t0, t1):
                nc.tensor.matmul(
                    box["pkv"][:, h0:h1],
                    wall_sb[:, t, H : 3 * H],
                    hidT_sb[:, 512 * c + h0 : 512 * c + h1, t],
                    start=(t == 0 and not zeroed),
                    stop=(t == NT - 1),
                    skip_group_check=True,
                )

        def k_add(box, c, h0=0, h1=512):
            nc.vector.tensor_scalar_add(
                kT[:, 512 * c + h0 : 512 * c + h1],
                box["pkv"][0:64, h0:h1],
                ball_sb[0:64, 0:1],
            )

        def v_pieces(box, c):
            def p_v():
                vstg = stage.tile([64, 512], F32R, tag="vstg")
                nc.vector.tensor_scalar_add(
                    vstg[:], box["pkv"][64:128, :], ball_sb[64:128, 0:1]
                )
                box["vstg"] = vstg

            def p_tp():
                pv = pjp.tile([128, 4, 64], F32R, tag="pj", name="pv")
                for j in range(4):
                    nc.tensor.transpose(
                        pv[:, j, :],
                        box["vstg"][:, 128 * j : 128 * (j + 1)],
                        identr[0:64, 0:64],
                    )
                box["pv"] = pv

            def p_tpc():
                nc.vector.tensor_copy(vones[:, 4 * c : 4 * c + 4, 0:64], box["pv"][:])

            return [p_v, p_tp, p_tpc]

        def kv_chunk_pieces(c):
            box = {}
            return [
                lambda: kv_mm(box, c, 0, 2),
                lambda: kv_mm(box, c, 2, 4),
                lambda: kv_mm(box, c, 4, 6),
                lambda: kv_mm(box, c, 6, 8),
                lambda: k_add(box, c),
            ] + v_pieces(box, c)

        # ---- startup: q group 0 + kv chunk 0 in 128-row quarters, each
        # gated only on its own hid sliver so PE tracks the DMA stream.
        # Chunk 0's v work (consumers are pass-1-deferred avs) moves into
        # the head's B-exp window.
        kv0 = {"z": False}
        qg0 = {"z": False}
        q_mm(qg0, 0, 0, 8, 0, 128)
        kv_mm(kv0, 0, 0, 8, 0, 128)
        k_add(kv0, 0, 0, 128)
        q_mm(qg0, 0, 0, 8, 128, 256)
        q_add(qg0, 0, 0, 256)
        kv_mm(kv0, 0, 0, 8, 128, 256)
        k_add(kv0, 0, 128, 256)
        q_mm(qg0, 0, 0, 8, 256, 384)
        q_mm(qg0, 0, 0, 8, 384, 512)
        q_add(qg0, 0, 256, 512)
        kv_mm(kv0, 0, 0, 8, 256, 512)
        k_add(kv0, 0, 256, 512)
        v0 = v_pieces(kv0, 0)

        # deferred pieces: kv chunks 2-7 then q groups 2/3 and chunk 7's
        # v pieces, popped 2/tile (one before the sc matmuls, one after
        # the exp) from tile HEAD_T, plus a 3rd pop in the tail (t>=26)
        # where avs are deferred.  Chunk c's k_add lands exactly at tile
        # 4c's pre-pop.  Everything drains inside pass 0: pass-1's
        # accumulators reuse pjp slots, so any piece left for pass 1
        # deadlocks the pjp rotation against their memsets.
        pieces = []
        for c in (2, 3, 4, 5, 6):
            pieces += kv_chunk_pieces(c)
        c7 = kv_chunk_pieces(7)
        pieces += c7[:5]
        pieces += q_group_pieces(2)
        pieces += q_group_pieces(3)
        pieces += c7[5:]

        def attn_pass(
            P, accs, interleave, head=False, tail=None, defer=False, extra=None
        ):
            """One sweep over all 32 key tiles for q columns [1024P, 1024P+1024).

            accs: {"A":..., "B":...} accumulator APs - pass 0 gets an empty
            dict filled at the pass boundary (its avs are all deferred, so
            the accp banks are free during pass 0 and serve as a THIRD
            score buffer, letting PE run 2 sc tiles ahead of ACT through
            the PE-oversubscribed piece window).
            """
            wts = {}
            scs = {}
            scn = [0]

            def sc_alloc(w, name):
                scn[0] += 1
                if P == 0 and scn[0] % 3 == 0 and scn[0] <= 36:
                    return accp.tile([128, w], F32, tag="acc", name=name + "x")
                return scp.tile([128, w], F32, tag="sc", name=name)

            def sc_mm(t, h, sc):
                nc.tensor.matmul(
                    sc[:],
                    kT[:, 128 * t : 128 * (t + 1)],
                    qT[:, 1024 * P + 512 * h : 1024 * P + 512 * (h + 1)],
                    start=True,
                    stop=True,
                )

            def piece():
                if interleave and pieces:
                    pieces.pop(0)()

            # pre-zero the acc banks on DVE (matmul start=True cannot be
            # used per-region: it wipes other open accumulations in the same
            # bank; a PE zeroing matmul also races the previous pass's
            # out_block reads), then accumulate in place with start=False.
            for k in ("A", "B"):
                if k in accs:
                    nc.vector.memset(accs[k][:], 0.0)

            def av_js(t, j0, j1):
                wt = wts[t]
                for j in range(j0, j1):
                    acc = accs["A"] if j < 4 else accs["B"]
                    nc.tensor.matmul(
                        acc[:, j % 4, 0:65],
                        wt[:, 128 * j : 128 * (j + 1)],
                        vones[:, t, :],
                        start=False,
                        stop=(t == NK - 1),
                        skip_group_check=True,
                    )

            def av(t):
                av_js(t, 0, 8)
                del wts[t]

            def scq(t, q0, q1):
                if t not in wts:
                    wts[t] = wtp.tile([128, 1024], BF16, tag="wt", name=f"wt{t}_{P}")
                half = sc_alloc(q1 - q0, f"scq{P}_{t}_{q0}")
                nc.tensor.matmul(
                    half[:],
                    kT[:, 128 * t : 128 * (t + 1)],
                    qT[:, 1024 * P + q0 : 1024 * P + q1],
                    start=True,
                    stop=True,
                )
                nc.scalar.activation(wts[t][:, q0:q1], half[:], Exp, scale=0.125)

            t0 = 0
            if head:
                # A quarters of tiles 0..3 need only qT[0:256] + kT quarters
                # so ACT starts ~6.7us in; chunk-0/1/qg1 projections
                # interleave at their DMA-ready points (PE dispatches
                # out-of-order within its 32-deep window, so deps rule);
                # chunk 0/1 v work and chunk 2's kv pieces fill the B-exp
                # window so kT(c2) is ready well before tile 8.
                c1 = {"z": False}
                qg1 = {"z": False}
                # ACT is strict FIFO: emit exps in gate-readiness order --
                # tiles 0/1 need only kT[0:256] (ready early), the A2
                # quarters of 0/1 beat tiles 2/3's kT[256:512] chain
                scq(0, 0, 256)
                scq(1, 0, 256)
                scq(0, 256, 512)
                scq(1, 256, 512)
                scq(2, 0, 256)
                scq(3, 0, 256)
                scq(2, 256, 512)
                scq(3, 256, 512)
                kv_mm(c1, 1, 0, 8, 0, 256)
                k_add(c1, 1, 0, 128)
                k_add(c1, 1, 128, 256)
                v0[0]()
                q_mm(qg1, 1, 0, 8, 0, 256)
                q_add(qg1, 1, 0, 256)
                scq(4, 0, 512)
                scq(5, 0, 512)
                q_mm(qg1, 1, 0, 8, 256, 512)
                q_add(qg1, 1, 256, 512)
                scq(0, 512, 1024)
                scq(1, 512, 1024)
                kv_mm(c1, 1, 0, 8, 256, 512)
                k_add(c1, 1, 256, 512)
                scq(2, 512, 1024)
                v0[1]()
                v0[2]()
                scq(3, 512, 1024)
                piece()
                for fn in v_pieces(c1, 1):
                    fn()
                scq(4, 512, 1024)
                piece()
                scq(5, 512, 1024)
                piece()
                t0 = HEAD_T

            for t in range(t0, NK):
                piece()
                if tail is not None and t == NK - 1:
                    # split the last exp so the A-half flush (j 0-3 reads
                    # only wt[:, 0:512]) starts half an exp earlier
                    scq(t, 0, 512)
                    scq(t, 512, 1024)
                else:
                    scs[t] = sc_alloc(1024, f"sc{t}_{P}")
                    nc.tensor.matmul(
                        scs[t][:, 0:512],
                        kT[:, 128 * t : 128 * (t + 1)],
                        qT[:, 1024 * P : 1024 * P + 512],
                        start=True,
                        stop=True,
                    )
                    nc.tensor.matmul(
                        scs[t][:, 512:1024],
                        kT[:, 128 * t : 128 * (t + 1)],
                        qT[:, 1024 * P + 512 : 1024 * P + 1024],
                        start=True,
                        stop=True,
                    )
                    wt = wtp.tile([128, 1024], BF16, tag="wt", name=f"wt{t}_{P}")
                    nc.scalar.activation(wt[:], scs[t][:], Exp, scale=0.125)
                    wts[t] = wt
                scs.pop(t - LAG, None)
                piece()
                if extra:
                    extra.pop(0)()
                if extra and t < 2:
                    extra.pop(0)()
                if t >= LAG and (not defer or t - LAG >= DEFER):
                    av(t - LAG)
                if t >= 28:
                    piece()
            if interleave:
                while pieces:
                    pieces.pop(0)()
            if defer:
                # defer the avs of the piece-congested window (tiles 0..15)
                # AND the final-tile flush into the next pass, which has PE
                # slack and no pieces (the flush would stall PE on this
                # pass's last exps right when the next pass's scores could
                # run).  Keep tiles 28..31 last so each region's stop flag
                # still closes its accumulation group.
                return [
                    (lambda tt: lambda: av(tt))(t)
                    for t in list(range(DEFER)) + list(range(NK - LAG, NK))
                ]
            if tail is None:
                for t in range(NK - LAG, NK):
                    av(t)
            else:
                # finish accA's accumulation first: its avs need only the
                # A half of the (split) last exp, so its normalize + DMA
                # overlap the B half's exp and flush
                cb_b, cb_a = tail
                for t in range(NK - LAG, NK):
                    av_js(t, 0, 4)
                cb_a()
                for t in range(NK - LAG, NK):
                    av_js(t, 4, 8)
                cb_b()

        def out_block(acc, blk, on_act=False):
            # blk in 0..3: output rows 512*blk .. 512*blk+512.  The final
            # (tail) blocks split the multiplies between ACT and DVE.
            res = stage.tile([128, 4, H], F32, tag="res")
            for j in range(4):
                rec = stage.tile([128, 1], F32, tag="rec")
                nc.vector.reciprocal(rec[:], acc[:, j, 64:65])
                if on_act:
                    nc.scalar.activation(
                        res[:, j, :], acc[:, j, 0:64], Copy, scale=rec[:]
                    )
                else:
                    nc.vector.tensor_scalar_mul(res[:, j, :], acc[:, j, 0:64], rec[:])
            nc.sync.dma_start(
                OUT[512 * blk : 512 * (blk + 1), :].rearrange("(j p) c -> p j c", p=128),
                res[:],
            )

        acc0 = {}
        av_defer = attn_pass(0, acc0, interleave=True, head=True, defer=True)
        # pass-0 accumulators materialize only now: one [128,8,128] tile
        # (A = slots 0:4, B = 4:8; each half stays inside one bank)
        accAB = accp.tile([128, 8, 128], F32, tag="acc", name="accAB")
        nc.vector.memset(accAB[:], 0.0)
        acc0["A"] = accAB[:, 0:4, :]
        acc0["B"] = accAB[:, 4:8, :]
        assert not pieces, f"{len(pieces)} deferred pieces never emitted"
        if dbg:
            qtd = dbgp.tile([64, SQ], F32, tag="qtd")
            nc.vector.tensor_copy(qtd[:], qT[:])
            nc.sync.dma_start(QTD[:], qtd[:])
            ktd = dbgp.tile([64, S], F32, tag="ktd")
            nc.vector.tensor_copy(ktd[:], kT[:])
            nc.sync.dma_start(KTD[:], ktd[:])
            vod = dbgp.tile([128, NK * (H + 1)], F32, tag="vod")
            nc.vector.tensor_copy(vod[:], vones[:].rearrange("p a b -> p (a b)"))
            nc.sync.dma_start(VOD[:], vod[:])
            accd = dbgp.tile([128, 4 * 128], F32, tag="accd")
            nc.vector.tensor_copy(accd[:], acc0["A"].rearrange("p a b -> p (a b)"))
            nc.sync.dma_start(ACCD[:], accd[:])
        av_defer.append(lambda: out_block(acc0["A"], 0))
        av_defer.append(lambda: out_block(acc0["B"], 1))

        # pass-1 accumulators live in the (now idle) pjp banks so pass 0's
        # stay valid while its deferred avs drain inside pass 1
        acc1A = pjp.tile([128, 4, 128], F32, tag="pj", name="acc1A")
        acc1B = pjp.tile([128, 4, 128], F32, tag="pj", name="acc1B")
        res8 = stage.tile([128, 8, H], F32, tag="res8", name="res8")

        def tail_half(acc, r0, dve_only):
            # one strided reciprocal for the half's 4 denominators, then
            # multiply pairs with the pair's output DMA issued immediately
            # (overlapping the DMA-issue pipeline with the remaining
            # normalize work).  The first (critical-path) half keeps
            # everything on DVE — one queue, no cross-engine semaphore
            # hops; the second half splits DVE/ACT for parallelism.
            rec4 = stage.tile([128, 4], F32, tag="rec4", name=f"rec4_{r0}")
            for j in range(4):
                nc.vector.reciprocal(rec4[:, j : j + 1], acc[:, j, 64:65])
            # one fused (acc * 1.0) * rec multiply for the whole half via a
            # stride-0 broadcast view of rec4 — replaces 4 latency-chained
            # per-block multiplies
            ra = rec4[:]
            rb = type(ra)(ra.tensor, ra.offset, [list(d) for d in ra.ap] + [[0, 64]])
            nc.vector.scalar_tensor_tensor(
                res8[:, r0 : r0 + 4, :],
                acc[:, :, 0:64],
                1.0,
                rb,
                mybir.AluOpType.mult,
                mybir.AluOpType.mult,
            )
            row = 1024 + 128 * r0
            nc.sync.dma_start(
                OUT[row : row + 512, :].rearrange("(j p) c -> p j c", p=128),
                res8[:, r0 : r0 + 4, :],
            )

        def tail_b():
            tail_half(acc1B, 4, dve_only=True)

        def tail_a():
            tail_half(acc1A, 0, dve_only=False)

        attn_pass(
            1,
            {"A": acc1A, "B": acc1B},
            interleave=False,
            extra=av_defer,
            tail=(tail_b, tail_a),
        )
        assert not av_defer, f"{len(av_defer)} deferred avs never emitted"

    nc.compile()
    return nc


def kernel(hidden_states, Wq, bq, Wk, bk, Wv, bv):
    global _NC, LAST_RESULT
    import ml_dtypes
    from concourse.bass_utils import run_bass_kernel_spmd

    BF = ml_dtypes.bfloat16
    hidden_states = np.asarray(hidden_states, dtype=np.float32)
    Wq = np.asarray(Wq, dtype=np.float32)
    Wk = np.asarray(Wk, dtype=np.float32)
    Wv = np.asarray(Wv, dtype=np.float32)
    bq = np.asarray(bq, dtype=np.float32)
    bk = np.asarray(bk, dtype=np.float32)
    bv = np.asarray(bv, dtype=np.float32)
    B = hidden_states.shape[0]
    assert hidden_states.shape == (4, S, E), hidden_states.shape

    if _NC is None:
        _NC = _build()

    wall = np.concatenate([Wq, Wk, Wv], axis=1)  # [E, 192]
    wall_t = np.ascontiguousarray(
        wall.reshape(NT, 128, 3 * H).transpose(1, 0, 2).astype(BF)
    )
    ball = np.zeros((128, 2), np.float32)
    ball[0:64, 0] = bk
    ball[64:128, 0] = bv
    ball[0:64, 1] = bq

    in_maps = []
    for core in range(N_CORES):
        b, half = divmod(core, 2)
        q0 = half * SQ
        hid_rot = np.roll(hidden_states[b], -q0, axis=0)  # [S, E]
        hidt = np.ascontiguousarray(
            hid_rot.reshape(S, NT, 128).transpose(2, 0, 1).astype(BF)
        )  # [128, S, NT]
        in_maps.append({"hidt": hidt, "wall": wall_t, "ball": ball})

    LAST_RESULT = run_bass_kernel_spmd(_NC, in_maps, core_ids=list(range(N_CORES)))
    out = np.empty((B, S, H), np.float32)
    for core in range(N_CORES):
        b, half = divmod(core, 2)
        q0 = half * SQ
        out[b, q0 : q0 + SQ] = LAST_RESULT.results[core]["out"]
    return out

